# revision 21
# baseline (speedup 1.0000x reference)
"""Trainium2 Bass kernel for an 8-expert top-2 MoE block (dim=1024, hidden=2048).

Strategy (expert-parallel, per the sharding hint):
  - Host computes the router top-2 dispatch (a sharding decision) and compacts
    each expert's tokens into a padded [dim, C] buffer, transposed so the
    device can run weight-stationary matmuls without any on-chip transposes.
  - Core e holds expert e's weights resident in SBUF and runs the 2-layer
    GELU MLP over its gathered tokens: x -> w1 -> gelu -> w2 (+biases).
  - The router/softmax statistics needed for the aux load-balancing loss are
    computed on-device data-parallel: core e processes token shard e
    (1024 tokens), producing per-expert softmax-prob sums and top-2 counts.
  - Host scatters the gated expert outputs back to token order and reduces
    the aux statistics to the scalar aux loss.
"""

import os
import sys
import types

import numpy as np

# Make sure jax can see the axon-tunneled NeuronCores if it hasn't loaded yet.
_jp = os.environ.get("JAX_PLATFORMS", "")
if "axon" not in _jp and "jax" not in sys.modules:
    os.environ["JAX_PLATFORMS"] = "axon,cpu"


def _install_axon_hook_shim():
    """Provide antenv.axon_hooks if the image's antenv predates it, so
    run_bass_kernel_spmd(trace=True) works instead of crashing."""
    try:
        import antenv.axon_hooks  # noqa: F401
        return
    except Exception:
        pass
    try:
        import antenv
    except Exception:
        return
    m = types.ModuleType("antenv.axon_hooks")
    m._h = None

    def set_axon_ntff_profile_hook(h):
        m._h = h

    def get_axon_ntff_profile_hook():
        return m._h

    m.set_axon_ntff_profile_hook = set_axon_ntff_profile_hook
    m.get_axon_ntff_profile_hook = get_axon_ntff_profile_hook
    sys.modules["antenv.axon_hooks"] = m
    antenv.axon_hooks = m
    try:
        from trn_agent_boot.trn_boot import _ntff_profile_via_ctypes

        h = _ntff_profile_via_ctypes("/opt/axon/libaxon_pjrt.so")
        if h is not None:
            m._h = h
    except Exception:
        pass


_install_axon_hook_shim()

import concourse.bass as bass  # noqa: E402
import concourse.mybir as mybir  # noqa: E402
from concourse import bacc  # noqa: E402
from concourse.tile import TileContext  # noqa: E402
from concourse.bass_utils import run_bass_kernel_spmd  # noqa: E402
from concourse.masks import make_identity  # noqa: E402

F32 = mybir.dt.float32
F32R = mybir.dt.float32r
AF = mybir.ActivationFunctionType
ALU = mybir.AluOpType
AX = mybir.AxisListType

B, S, DIM, HIDDEN = 2, 4096, 1024, 2048
T = B * S           # 8192 tokens
E = 8               # experts == cores
TOPK = 2
P = 128
DB = DIM // P       # 8 blocks of 128 along dim
HB = HIDDEN // P    # 16 blocks of 128 along hidden
TSH = T // E        # 1024 tokens per core for the aux router shard
TB = TSH // P       # 8 token blocks per shard

T_TILE = 512        # tokens per MLP pass (fp32 moving-dim max)

N_CORES = 8

# Stash of the most recent device-run results (exec_time_ns etc.) for test
# harnesses; not used by kernel() itself.
LAST_RESULTS = None


def _build(C, mm_dt=F32R, act_fn=None):
    """Build the SPMD Bass program for capacity C (multiple of 256)."""
    if act_fn is None:
        act_fn = AF.Gelu
    assert C % 256 == 0
    # Full tiles first (PE starts dense and ramps once); remainder last so
    # the final output DMA + drain tail is small.
    rem = C % T_TILE
    widths = [T_TILE] * (C // T_TILE) + ([rem] if rem else [])
    tiles = []
    t0 = 0
    for tw in widths:
        tiles.append((t0, tw))
        t0 += tw
    # Bacc (not raw Bass): its compile() pipeline runs
    # move_matmul_waits_to_ldweights + generate_event_semaphores, which split
    # sync waits down to the 1-wait-per-instruction limit of this walrus.
    nc = bacc.Bacc(None, target_bir_lowering=False)

    xgt = nc.dram_tensor("xgt", [DIM, C], mm_dt, kind="ExternalInput")
    w1t = nc.dram_tensor("w1t", [DIM, HIDDEN], mm_dt, kind="ExternalInput")
    w2t = nc.dram_tensor("w2t", [HIDDEN, DIM], mm_dt, kind="ExternalInput")
    b1r = nc.dram_tensor("b1r", [P, HB], F32, kind="ExternalInput")
    b2r = nc.dram_tensor("b2r", [P, DB], F32, kind="ExternalInput")
    xst = nc.dram_tensor("xst", [DIM, TSH], mm_dt, kind="ExternalInput")
    rwt = nc.dram_tensor("rwt", [DIM, E], mm_dt, kind="ExternalInput")
    y = nc.dram_tensor("y", [DIM, C], F32, kind="ExternalOutput")
    aux = nc.dram_tensor("aux", [1, 2 * E], F32, kind="ExternalOutput")

    xgt_r = xgt.rearrange("(a p) t -> p a t", p=P)
    w1t_r = w1t.rearrange("(a p) h -> p a h", p=P)
    w2t_r = w2t.rearrange("(a p) d -> p a d", p=P)
    xst_r = xst.rearrange("(a p) t -> p a t", p=P)
    rwt_r = rwt.rearrange("(a p) e -> p a e", p=P)
    y_r = y.rearrange("(a p) t -> p a t", p=P)

    RT = 256               # router token tile (moving dim)
    NRT = TSH // RT        # router tiles per shard

    # All input loads go on the SP HWDGE ring (FIFO per ring, each transfer
    # striped across all 16 SDMA engines) in the order the PE needs them;
    # outputs go on the ACT ring so they never block input loads.
    with TileContext(nc) as tc:
        with (
            tc.tile_pool(name="wsb", bufs=1) as wsb,
            tc.tile_pool(name="csb", bufs=1) as csb,
            tc.tile_pool(name="xsb", bufs=2) as xsb,
            tc.tile_pool(name="hsb", bufs=1) as hsb,
            tc.tile_pool(name="osb", bufs=2) as osb,
            tc.tile_pool(name="rsb", bufs=2) as rsb,
            tc.tile_pool(name="xssb", bufs=1) as xssb,
        ):
            b1s = csb.tile([P, HB], F32)
            nc.sync.dma_start(out=b1s, in_=b1r[:, :])
            b2s = csb.tile([P, DB], F32)
            nc.sync.dma_start(out=b2s, in_=b2r[:, :])

            def load_xg(t0, tw):
                xg = []
                for db in range(DB):
                    xt = xsb.tile([P, tw], mm_dt, name=f"xg{db}", tag=f"xg{db}")
                    nc.sync.dma_start(out=xt, in_=xgt_r[:, db, t0 : t0 + tw])
                    xg.append(xt)
                return xg

            def l1(xg, tw):
                h = hsb.tile([P, HB, tw], mm_dt, name="h", tag="h")
                for hb in range(HB):
                    pp = ps1.tile([P, tw], F32, name="pp1", tag="pp1")
                    for db in range(DB):
                        nc.tensor.matmul(
                            pp,
                            lhsT=w1s[db][:, hb * P : (hb + 1) * P],
                            rhs=xg[db],
                            start=(db == 0),
                            stop=(db == DB - 1),
                        )
                    nc.scalar.activation(
                        h[:, hb, :], pp, act_fn, bias=b1s[:, hb : hb + 1]
                    )
                return h

            def l2(h, t0, tw):
                for db in range(DB):
                    pp = ps2.tile([P, tw], F32, name="pp2", tag="pp2")
                    for hb in range(HB):
                        nc.tensor.matmul(
                            pp,
                            lhsT=w2s[hb][:, db * P : (db + 1) * P],
                            rhs=h[:, hb, :],
                            start=(hb == 0),
                            stop=(hb == HB - 1),
                        )
                    ot = osb.tile([P, tw], F32, name="ot", tag="ot")
                    nc.vector.tensor_scalar(
                        ot, pp, b2s[:, db : db + 1], None, op0=ALU.add
                    )
                    nc.scalar.dma_start(out=y_r[:, db, t0 : t0 + tw], in_=ot)

            # tile 0 activations first on the ring, then w1 in the exact
            # order layer 1 consumes it (hb block 0 first at fine grain).
            xg0 = load_xg(*tiles[0])
            w1s = [wsb.tile([P, HIDDEN], mm_dt, name=f"w1s{db}") for db in range(DB)]
            QW = 512
            for q in range(HIDDEN // QW):
                cs = slice(q * QW, (q + 1) * QW)
                for db in range(DB):
                    nc.sync.dma_start(out=w1s[db][:, cs], in_=w1t_r[:, db, cs])

            # router constants + state (aux phase B accumulators)
            ones = csb.tile([P, 1], F32)
            nc.vector.memset(ones, 1.0)
            ident = csb.tile([E, E], F32)
            make_identity(nc, ident)
            rws = csb.tile([P, DB, E], mm_dt)
            nc.sync.dma_start(out=rws, in_=rwt_r)
            lgs = csb.tile([P, TB, E], F32)
            accP = csb.tile([P, E], F32)
            nc.vector.memset(accP, 0.0)
            accF = csb.tile([P, E], F32)
            nc.vector.memset(accF, 0.0)

            with tc.tile_pool(name="ps1", bufs=4, space="PSUM") as ps1:
                h0 = l1(xg0, tiles[0][1])

                # ---- aux router phase A: logits + transposes (PE), stashed
                # to SBUF; xsT loads queue behind w1 on the ring. ----
                with (
                    tc.tile_pool(name="rps", bufs=2, space="PSUM") as rps,
                    tc.tile_pool(name="rtps", bufs=2, space="PSUM") as rtps,
                ):
                    nblk = 0
                    for rt in range(NRT):
                        xs = []
                        for db in range(DB):
                            xsd = xssb.tile(
                                [P, RT], mm_dt, name=f"xs{db}", tag=f"xs{db}"
                            )
                            nc.sync.dma_start(
                                out=xsd,
                                in_=xst_r[:, db, rt * RT : (rt + 1) * RT],
                            )
                            xs.append(xsd)
                        ltp = rps.tile([E, RT], F32, name="ltp")
                        for db in range(DB):
                            nc.tensor.matmul(
                                ltp,
                                lhsT=rws[:, db, :],
                                rhs=xs[db],
                                start=(db == 0),
                                stop=(db == DB - 1),
                            )
                        lt = rsb.tile([E, RT], F32, name="lt")
                        nc.vector.tensor_copy(lt, ltp)
                        for j in range(RT // P):
                            lg = rtps.tile([P, E], F32, name="lg")
                            nc.tensor.transpose(
                                lg, lt[:, j * P : (j + 1) * P], ident
                            )
                            nc.vector.tensor_copy(lgs[:, nblk, :], lg)
                            nblk += 1

                # w2, in the half-major order layer 2 consumes it.
                w2s = [wsb.tile([P, DIM], mm_dt, name=f"w2s{hb}") for hb in range(HB)]
                for half in range(2):
                    cs = slice(half * (DIM // 2), (half + 1) * (DIM // 2))
                    for hb in range(HB):
                        nc.sync.dma_start(out=w2s[hb][:, cs], in_=w2t_r[:, hb, cs])

                with (
                    tc.tile_pool(name="ps2", bufs=3, space="PSUM") as ps2,
                    tc.tile_pool(name="accps", bufs=1, space="PSUM") as accps,
                ):
                    l2(h0, tiles[0][0], tiles[0][1])

                    # ---- aux router phase B: softmax/top-2 on DVE/ACT,
                    # accumulated on DVE; overlaps the MLP's PE stream. ----
                    for blk in range(TB):
                        lg = lgs[:, blk, :]
                        mx = rsb.tile([P, 8], F32, name="mx")
                        nc.vector.max(out=mx, in_=lg)
                        nm = rsb.tile([P, 1], F32, name="nm")
                        nc.vector.tensor_scalar_mul(nm, mx[:, 0:1], -1.0)
                        ex = rsb.tile([P, E], F32, name="ex")
                        nc.scalar.activation(ex, lg, AF.Exp, bias=nm)
                        sm = rsb.tile([P, 1], F32, name="sm")
                        nc.vector.reduce_sum(sm, ex, axis=AX.X)
                        rs = rsb.tile([P, 1], F32, name="rs")
                        nc.vector.reciprocal(rs, sm)
                        pr = rsb.tile([P, E], F32, name="pr")
                        nc.vector.tensor_mul(pr, ex, rs.to_broadcast([P, E]))
                        nc.vector.tensor_add(accP, accP, pr)
                        msk = rsb.tile([P, E], F32, name="msk")
                        nc.vector.tensor_tensor(
                            msk,
                            lg,
                            mx[:, 1:2].to_broadcast([P, E]),
                            op=ALU.is_ge,
                        )
                        nc.vector.tensor_add(accF, accF, msk)
                    pPF = accps.tile([1, 2 * E], F32, name="pPF")
                    nc.tensor.matmul(
                        pPF[:, 0:E], lhsT=ones, rhs=accP,
                        start=True, stop=True,
                    )
                    nc.tensor.matmul(
                        pPF[:, E : 2 * E], lhsT=ones, rhs=accF,
                        start=True, stop=True, skip_group_check=True,
                    )
                    auxs = rsb.tile([1, 2 * E], F32, name="auxs")
                    nc.vector.tensor_copy(auxs, pPF)
                    nc.scalar.dma_start(out=aux[:, :], in_=auxs)

                    for t0, tw in tiles[1:]:
                        xg = load_xg(t0, tw)
                        h = l1(xg, tw)
                        l2(h, t0, tw)

    nc.finalize()
    return nc


def _route_host(xf, router_w):
    """Host top-2 routing (the sharding decision). Matches jax.lax.top_k
    tie-breaking (lowest index wins)."""
    logits = xf @ router_w.T  # [T, E] f32
    t_idx = np.arange(logits.shape[0])
    i1 = np.argmax(logits, axis=1)
    l1 = logits[t_idx, i1]
    lm = logits.copy()
    lm[t_idx, i1] = -np.inf
    i2 = np.argmax(lm, axis=1)
    l2 = logits[t_idx, i2]
    # softmax over the two selected logits (l1 >= l2)
    e2 = np.exp((l2 - l1).astype(np.float32))
    g1 = (1.0 / (1.0 + e2)).astype(np.float32)
    g2 = (e2 / (1.0 + e2)).astype(np.float32)
    return i1, i2, g1, g2


def kernel(x, router_w, w1, b1, w2, b2):
    global LAST_RESULTS
    x = np.asarray(x, dtype=np.float32)
    router_w = np.asarray(router_w, dtype=np.float32)
    w1 = np.asarray(w1, dtype=np.float32)
    b1 = np.asarray(b1, dtype=np.float32)
    w2 = np.asarray(w2, dtype=np.float32)
    b2 = np.asarray(b2, dtype=np.float32)

    xf = x.reshape(T, DIM)
    i1, i2, g1, g2 = _route_host(xf, router_w)

    idx = []
    gates = []
    for e in range(E):
        sel1 = i1 == e
        sel2 = i2 == e
        ie = np.where(sel1 | sel2)[0]
        ge = np.where(sel1[ie], g1[ie], g2[ie]).astype(np.float32)
        idx.append(ie)
        gates.append(ge)

    max_n = max(len(ie) for ie in idx)
    C = max(256, ((max_n + 255) // 256) * 256)

    mm_dt = F32 if os.environ.get("MOE_MM_F32") else F32R
    nc = _build(C, mm_dt=mm_dt)

    rwt = np.ascontiguousarray(router_w.T)
    in_maps = []
    for e in range(E):
        xgt = np.zeros((DIM, C), dtype=np.float32)
        ie = idx[e]
        if len(ie):
            xgt[:, : len(ie)] = xf[ie].T
        in_maps.append(
            {
                "xgt": xgt,
                "w1t": np.ascontiguousarray(w1[e].T),
                "w2t": np.ascontiguousarray(w2[e].T),
                "b1r": np.ascontiguousarray(b1[e].reshape(HB, P).T),
                "b2r": np.ascontiguousarray(b2[e].reshape(DB, P).T),
                "xst": np.ascontiguousarray(xf[e * TSH : (e + 1) * TSH].T),
                "rwt": rwt,
            }
        )

    trace = bool(os.environ.get("BASS_TRACE"))
    res = run_bass_kernel_spmd(
        nc,
        in_maps,
        core_ids=list(range(N_CORES)),
        trace=trace,
    )
    LAST_RESULTS = res

    out = np.zeros((T, DIM), dtype=np.float32)
    p_sum = np.zeros(E, dtype=np.float64)
    f_cnt = np.zeros(E, dtype=np.float64)
    for e in range(E):
        r = res.results[e]
        ie = idx[e]
        ye = r["y"]
        out[ie] += gates[e][:, None] * ye[:, : len(ie)].T
        p_sum += r["aux"][0, :E].astype(np.float64)
        f_cnt += r["aux"][0, E:].astype(np.float64)

    P_i = p_sum / T
    f_i = f_cnt / (T * TOPK)
    aux_loss = np.float32(E * np.sum(f_i * P_i))

    return out.reshape(B, S, DIM), aux_loss


# revision 23
# speedup vs baseline: 1.0090x; 1.0090x over previous
"""Trainium2 Bass kernel for an 8-expert top-2 MoE block (dim=1024, hidden=2048).

Strategy (expert-parallel, per the sharding hint):
  - Host computes the router top-2 dispatch (a sharding decision) and compacts
    each expert's tokens into a padded [dim, C] buffer, transposed so the
    device can run weight-stationary matmuls without any on-chip transposes.
  - Core e holds expert e's weights resident in SBUF and runs the 2-layer
    GELU MLP over its gathered tokens: x -> w1 -> gelu -> w2 (+biases).
  - The router/softmax statistics needed for the aux load-balancing loss are
    computed on-device data-parallel: core e processes token shard e
    (1024 tokens), producing per-expert softmax-prob sums and top-2 counts.
  - Host scatters the gated expert outputs back to token order and reduces
    the aux statistics to the scalar aux loss.
"""

import os
import sys
import types

import numpy as np

# Make sure jax can see the axon-tunneled NeuronCores if it hasn't loaded yet.
_jp = os.environ.get("JAX_PLATFORMS", "")
if "axon" not in _jp and "jax" not in sys.modules:
    os.environ["JAX_PLATFORMS"] = "axon,cpu"


def _install_axon_hook_shim():
    """Provide antenv.axon_hooks if the image's antenv predates it, so
    run_bass_kernel_spmd(trace=True) works instead of crashing."""
    try:
        import antenv.axon_hooks  # noqa: F401
        return
    except Exception:
        pass
    try:
        import antenv
    except Exception:
        return
    m = types.ModuleType("antenv.axon_hooks")
    m._h = None

    def set_axon_ntff_profile_hook(h):
        m._h = h

    def get_axon_ntff_profile_hook():
        return m._h

    m.set_axon_ntff_profile_hook = set_axon_ntff_profile_hook
    m.get_axon_ntff_profile_hook = get_axon_ntff_profile_hook
    sys.modules["antenv.axon_hooks"] = m
    antenv.axon_hooks = m
    try:
        from trn_agent_boot.trn_boot import _ntff_profile_via_ctypes

        h = _ntff_profile_via_ctypes("/opt/axon/libaxon_pjrt.so")
        if h is not None:
            m._h = h
    except Exception:
        pass


_install_axon_hook_shim()

import concourse.bass as bass  # noqa: E402
import concourse.mybir as mybir  # noqa: E402
from concourse import bacc  # noqa: E402
from concourse.tile import TileContext  # noqa: E402
from concourse.bass_utils import run_bass_kernel_spmd  # noqa: E402
from concourse.masks import make_identity  # noqa: E402

F32 = mybir.dt.float32
F32R = mybir.dt.float32r
AF = mybir.ActivationFunctionType
ALU = mybir.AluOpType
AX = mybir.AxisListType

B, S, DIM, HIDDEN = 2, 4096, 1024, 2048
T = B * S           # 8192 tokens
E = 8               # experts == cores
TOPK = 2
P = 128
DB = DIM // P       # 8 blocks of 128 along dim
HB = HIDDEN // P    # 16 blocks of 128 along hidden
TSH = T // E        # 1024 tokens per core for the aux router shard
TB = TSH // P       # 8 token blocks per shard

T_TILE = 512        # tokens per MLP pass (fp32 moving-dim max)

N_CORES = 8

# Stash of the most recent device-run results (exec_time_ns etc.) for test
# harnesses; not used by kernel() itself.
LAST_RESULTS = None


def _build(C, mm_dt=F32R, act_fn=None, zero_bias=False):
    """Build the SPMD Bass program for capacity C (multiple of 256).

    zero_bias: both b1 and b2 are all-zero (checked by the caller), so skip
    the bias inputs entirely — drops two slow strided DMAs and the gelu's
    dependency on them."""
    if act_fn is None:
        act_fn = AF.Gelu
    assert C % 256 == 0
    # Full tiles first (PE starts dense and ramps once); remainder last so
    # the final output DMA + drain tail is small.
    rem = C % T_TILE
    widths = [T_TILE] * (C // T_TILE) + ([rem] if rem else [])
    tiles = []
    t0 = 0
    for tw in widths:
        tiles.append((t0, tw))
        t0 += tw
    # Bacc (not raw Bass): its compile() pipeline runs
    # move_matmul_waits_to_ldweights + generate_event_semaphores, which split
    # sync waits down to the 1-wait-per-instruction limit of this walrus.
    nc = bacc.Bacc(None, target_bir_lowering=False)

    xgt = nc.dram_tensor("xgt", [DIM, C], mm_dt, kind="ExternalInput")
    w1t = nc.dram_tensor("w1t", [DIM, HIDDEN], mm_dt, kind="ExternalInput")
    w2t = nc.dram_tensor("w2t", [HIDDEN, DIM], mm_dt, kind="ExternalInput")
    if not zero_bias:
        b1r = nc.dram_tensor("b1r", [P, HB], F32, kind="ExternalInput")
        b2r = nc.dram_tensor("b2r", [P, DB], F32, kind="ExternalInput")
    xst = nc.dram_tensor("xst", [DIM, TSH], mm_dt, kind="ExternalInput")
    rwt = nc.dram_tensor("rwt", [DIM, E], mm_dt, kind="ExternalInput")
    y = nc.dram_tensor("y", [DIM, C], F32, kind="ExternalOutput")
    aux = nc.dram_tensor("aux", [1, 2 * E], F32, kind="ExternalOutput")

    xgt_r = xgt.rearrange("(a p) t -> p a t", p=P)
    w1t_r = w1t.rearrange("(a p) h -> p a h", p=P)
    w2t_r = w2t.rearrange("(a p) d -> p a d", p=P)
    xst_r = xst.rearrange("(a p) t -> p a t", p=P)
    rwt_r = rwt.rearrange("(a p) e -> p a e", p=P)
    y_r = y.rearrange("(a p) t -> p a t", p=P)

    RT = 512               # router token tile (moving dim)
    NRT = TSH // RT        # router tiles per shard

    # All input loads go on the SP HWDGE ring (FIFO per ring, each transfer
    # striped across all 16 SDMA engines) in the order the PE needs them;
    # outputs go on the ACT ring so they never block input loads.
    with TileContext(nc) as tc:
        with (
            tc.tile_pool(name="wsb", bufs=1) as wsb,
            tc.tile_pool(name="csb", bufs=1) as csb,
            tc.tile_pool(name="xsb", bufs=2) as xsb,
            tc.tile_pool(name="hsb", bufs=1) as hsb,
            tc.tile_pool(name="osb", bufs=3) as osb,
            tc.tile_pool(name="rsb", bufs=2) as rsb,
        ):
            if not zero_bias:
                b1s = csb.tile([P, HB], F32)
                nc.sync.dma_start(out=b1s, in_=b1r[:, :])
                b2s = csb.tile([P, DB], F32)
                nc.sync.dma_start(out=b2s, in_=b2r[:, :])

            def load_xg(t0, tw):
                xg = []
                for db in range(DB):
                    xt = xsb.tile([P, tw], mm_dt, name=f"xg{db}", tag=f"xg{db}")
                    nc.sync.dma_start(out=xt, in_=xgt_r[:, db, t0 : t0 + tw])
                    xg.append(xt)
                return xg

            def l1(xg, tw):
                h = hsb.tile([P, HB, tw], mm_dt, name="h", tag="h")
                for hb in range(HB):
                    pp = ps1.tile([P, tw], F32, name="pp1", tag="pp1")
                    for db in range(DB):
                        nc.tensor.matmul(
                            pp,
                            lhsT=w1s[db][:, hb * P : (hb + 1) * P],
                            rhs=xg[db],
                            start=(db == 0),
                            stop=(db == DB - 1),
                        )
                    nc.scalar.activation(
                        h[:, hb, :], pp, act_fn,
                        bias=0.0 if zero_bias else b1s[:, hb : hb + 1],
                    )
                return h

            def l2(h, t0, tw):
                for db in range(DB):
                    pp = ps2.tile([P, tw], F32, name="pp2", tag="pp2")
                    for hb in range(HB):
                        nc.tensor.matmul(
                            pp,
                            lhsT=w2s[hb][:, db * P : (db + 1) * P],
                            rhs=h[:, hb, :],
                            start=(hb == 0),
                            stop=(hb == HB - 1),
                        )
                    ot = osb.tile([P, tw], F32, name="ot", tag="ot")
                    if zero_bias:
                        nc.vector.tensor_copy(ot, pp)
                    else:
                        nc.vector.tensor_scalar(
                            ot, pp, b2s[:, db : db + 1], None, op0=ALU.add
                        )
                    nc.scalar.dma_start(out=y_r[:, db, t0 : t0 + tw], in_=ot)

            # tile 0 activations first on the ring, then w1 in the exact
            # order layer 1 consumes it (hb block 0 first at fine grain).
            xg0 = load_xg(*tiles[0])
            w1s = [wsb.tile([P, HIDDEN], mm_dt, name=f"w1s{db}") for db in range(DB)]
            QW = 512
            for q in range(HIDDEN // QW):
                cs = slice(q * QW, (q + 1) * QW)
                for db in range(DB):
                    nc.sync.dma_start(out=w1s[db][:, cs], in_=w1t_r[:, db, cs])

            # router constants + state (aux phase B accumulators)
            ones = csb.tile([P, 1], F32)
            nc.vector.memset(ones, 1.0)
            ident = csb.tile([E, E], F32)
            make_identity(nc, ident)
            rws = csb.tile([P, DB, E], mm_dt)
            nc.sync.dma_start(out=rws, in_=rwt_r)
            lgs = csb.tile([P, TB, E], F32)
            accP = csb.tile([P, E], F32)
            nc.vector.memset(accP, 0.0)
            accF = csb.tile([P, E], F32)
            nc.vector.memset(accF, 0.0)

            with tc.tile_pool(name="ps1", bufs=4, space="PSUM") as ps1:
                h0 = l1(xg0, tiles[0][1])

                # ---- aux router phase A: logits + transposes (PE), stashed
                # to SBUF; xsT loads queue behind w1 on the ring. ----
                with (
                    tc.tile_pool(name="rps", bufs=2, space="PSUM") as rps,
                    tc.tile_pool(name="rtps", bufs=2, space="PSUM") as rtps,
                ):
                    nblk = 0
                    for rt in range(NRT):
                        xs = []
                        for db in range(DB):
                            xsd = xsb.tile(
                                [P, RT], mm_dt, name=f"xs{db}", tag=f"xg{db}"
                            )
                            nc.sync.dma_start(
                                out=xsd,
                                in_=xst_r[:, db, rt * RT : (rt + 1) * RT],
                            )
                            xs.append(xsd)
                        ltp = rps.tile([E, RT], F32, name="ltp")
                        for db in range(DB):
                            nc.tensor.matmul(
                                ltp,
                                lhsT=rws[:, db, :],
                                rhs=xs[db],
                                start=(db == 0),
                                stop=(db == DB - 1),
                            )
                        lt = rsb.tile([E, RT], F32, name="lt")
                        nc.vector.tensor_copy(lt, ltp)
                        for j in range(RT // P):
                            lg = rtps.tile([P, E], F32, name="lg")
                            nc.tensor.transpose(
                                lg, lt[:, j * P : (j + 1) * P], ident
                            )
                            nc.vector.tensor_copy(lgs[:, nblk, :], lg)
                            nblk += 1

                # w2, in the half-major order layer 2 consumes it.
                w2s = [wsb.tile([P, DIM], mm_dt, name=f"w2s{hb}") for hb in range(HB)]
                for half in range(2):
                    cs = slice(half * (DIM // 2), (half + 1) * (DIM // 2))
                    for hb in range(HB):
                        nc.sync.dma_start(out=w2s[hb][:, cs], in_=w2t_r[:, hb, cs])

                with (
                    tc.tile_pool(name="ps2", bufs=3, space="PSUM") as ps2,
                    tc.tile_pool(name="accps", bufs=1, space="PSUM") as accps,
                ):
                    l2(h0, tiles[0][0], tiles[0][1])

                    # ---- aux router phase B: softmax/top-2 on DVE/ACT,
                    # accumulated on DVE; overlaps the MLP's PE stream. ----
                    for blk in range(TB):
                        lg = lgs[:, blk, :]
                        mx = rsb.tile([P, 8], F32, name="mx")
                        nc.vector.max(out=mx, in_=lg)
                        nm = rsb.tile([P, 1], F32, name="nm")
                        nc.vector.tensor_scalar_mul(nm, mx[:, 0:1], -1.0)
                        ex = rsb.tile([P, E], F32, name="ex")
                        nc.scalar.activation(ex, lg, AF.Exp, bias=nm)
                        sm = rsb.tile([P, 1], F32, name="sm")
                        nc.vector.reduce_sum(sm, ex, axis=AX.X)
                        rs = rsb.tile([P, 1], F32, name="rs")
                        nc.vector.reciprocal(rs, sm)
                        pr = rsb.tile([P, E], F32, name="pr")
                        nc.vector.tensor_mul(pr, ex, rs.to_broadcast([P, E]))
                        nc.vector.tensor_add(accP, accP, pr)
                        msk = rsb.tile([P, E], F32, name="msk")
                        nc.vector.tensor_tensor(
                            msk,
                            lg,
                            mx[:, 1:2].to_broadcast([P, E]),
                            op=ALU.is_ge,
                        )
                        nc.vector.tensor_add(accF, accF, msk)
                    pPF = accps.tile([1, 2 * E], F32, name="pPF")
                    nc.tensor.matmul(
                        pPF[:, 0:E], lhsT=ones, rhs=accP,
                        start=True, stop=True,
                    )
                    nc.tensor.matmul(
                        pPF[:, E : 2 * E], lhsT=ones, rhs=accF,
                        start=True, stop=True, skip_group_check=True,
                    )
                    auxs = rsb.tile([1, 2 * E], F32, name="auxs")
                    nc.vector.tensor_copy(auxs, pPF)
                    nc.scalar.dma_start(out=aux[:, :], in_=auxs)

                    for t0, tw in tiles[1:]:
                        xg = load_xg(t0, tw)
                        h = l1(xg, tw)
                        l2(h, t0, tw)

    nc.finalize()
    return nc


def _route_host(xf, router_w):
    """Host top-2 routing (the sharding decision). Matches jax.lax.top_k
    tie-breaking (lowest index wins)."""
    logits = xf @ router_w.T  # [T, E] f32
    t_idx = np.arange(logits.shape[0])
    i1 = np.argmax(logits, axis=1)
    l1 = logits[t_idx, i1]
    lm = logits.copy()
    lm[t_idx, i1] = -np.inf
    i2 = np.argmax(lm, axis=1)
    l2 = logits[t_idx, i2]
    # softmax over the two selected logits (l1 >= l2)
    e2 = np.exp((l2 - l1).astype(np.float32))
    g1 = (1.0 / (1.0 + e2)).astype(np.float32)
    g2 = (e2 / (1.0 + e2)).astype(np.float32)
    return i1, i2, g1, g2


def kernel(x, router_w, w1, b1, w2, b2):
    global LAST_RESULTS
    x = np.asarray(x, dtype=np.float32)
    router_w = np.asarray(router_w, dtype=np.float32)
    w1 = np.asarray(w1, dtype=np.float32)
    b1 = np.asarray(b1, dtype=np.float32)
    w2 = np.asarray(w2, dtype=np.float32)
    b2 = np.asarray(b2, dtype=np.float32)

    xf = x.reshape(T, DIM)
    i1, i2, g1, g2 = _route_host(xf, router_w)

    idx = []
    gates = []
    for e in range(E):
        sel1 = i1 == e
        sel2 = i2 == e
        ie = np.where(sel1 | sel2)[0]
        ge = np.where(sel1[ie], g1[ie], g2[ie]).astype(np.float32)
        idx.append(ie)
        gates.append(ge)

    max_n = max(len(ie) for ie in idx)
    C = max(256, ((max_n + 255) // 256) * 256)

    mm_dt = F32 if os.environ.get("MOE_MM_F32") else F32R
    zero_bias = not (b1.any() or b2.any())
    nc = _build(C, mm_dt=mm_dt, zero_bias=zero_bias)

    rwt = np.ascontiguousarray(router_w.T)
    in_maps = []
    for e in range(E):
        xgt = np.zeros((DIM, C), dtype=np.float32)
        ie = idx[e]
        if len(ie):
            xgt[:, : len(ie)] = xf[ie].T
        im = {
            "xgt": xgt,
            "w1t": np.ascontiguousarray(w1[e].T),
            "w2t": np.ascontiguousarray(w2[e].T),
            "xst": np.ascontiguousarray(xf[e * TSH : (e + 1) * TSH].T),
            "rwt": rwt,
        }
        if not zero_bias:
            im["b1r"] = np.ascontiguousarray(b1[e].reshape(HB, P).T)
            im["b2r"] = np.ascontiguousarray(b2[e].reshape(DB, P).T)
        in_maps.append(im)

    trace = bool(os.environ.get("BASS_TRACE"))
    res = run_bass_kernel_spmd(
        nc,
        in_maps,
        core_ids=list(range(N_CORES)),
        trace=trace,
    )
    LAST_RESULTS = res

    out = np.zeros((T, DIM), dtype=np.float32)
    p_sum = np.zeros(E, dtype=np.float64)
    f_cnt = np.zeros(E, dtype=np.float64)
    for e in range(E):
        r = res.results[e]
        ie = idx[e]
        ye = r["y"]
        out[ie] += gates[e][:, None] * ye[:, : len(ie)].T
        p_sum += r["aux"][0, :E].astype(np.float64)
        f_cnt += r["aux"][0, E:].astype(np.float64)

    P_i = p_sum / T
    f_i = f_cnt / (T * TOPK)
    aux_loss = np.float32(E * np.sum(f_i * P_i))

    return out.reshape(B, S, DIM), aux_loss


# revision 24
# speedup vs baseline: 1.0308x; 1.0217x over previous
"""Trainium2 Bass kernel for an 8-expert top-2 MoE block (dim=1024, hidden=2048).

Strategy (expert-parallel, per the sharding hint):
  - Host computes the router top-2 dispatch (a sharding decision) and compacts
    each expert's tokens into a padded [dim, C] buffer, transposed so the
    device can run weight-stationary matmuls without any on-chip transposes.
  - Core e holds expert e's weights resident in SBUF and runs the 2-layer
    GELU MLP over its gathered tokens: x -> w1 -> gelu -> w2 (+biases).
  - The router/softmax statistics needed for the aux load-balancing loss are
    computed on-device data-parallel: core e processes token shard e
    (1024 tokens), producing per-expert softmax-prob sums and top-2 counts.
  - Host scatters the gated expert outputs back to token order and reduces
    the aux statistics to the scalar aux loss.
"""

import os
import sys
import types

import numpy as np

# Make sure jax can see the axon-tunneled NeuronCores if it hasn't loaded yet.
_jp = os.environ.get("JAX_PLATFORMS", "")
if "axon" not in _jp and "jax" not in sys.modules:
    os.environ["JAX_PLATFORMS"] = "axon,cpu"


def _install_axon_hook_shim():
    """Provide antenv.axon_hooks if the image's antenv predates it, so
    run_bass_kernel_spmd(trace=True) works instead of crashing."""
    try:
        import antenv.axon_hooks  # noqa: F401
        return
    except Exception:
        pass
    try:
        import antenv
    except Exception:
        return
    m = types.ModuleType("antenv.axon_hooks")
    m._h = None

    def set_axon_ntff_profile_hook(h):
        m._h = h

    def get_axon_ntff_profile_hook():
        return m._h

    m.set_axon_ntff_profile_hook = set_axon_ntff_profile_hook
    m.get_axon_ntff_profile_hook = get_axon_ntff_profile_hook
    sys.modules["antenv.axon_hooks"] = m
    antenv.axon_hooks = m
    try:
        from trn_agent_boot.trn_boot import _ntff_profile_via_ctypes

        h = _ntff_profile_via_ctypes("/opt/axon/libaxon_pjrt.so")
        if h is not None:
            m._h = h
    except Exception:
        pass


_install_axon_hook_shim()

import concourse.bass as bass  # noqa: E402
import concourse.mybir as mybir  # noqa: E402
from concourse import bacc  # noqa: E402
from concourse.tile import TileContext  # noqa: E402
from concourse.bass_utils import run_bass_kernel_spmd  # noqa: E402
from concourse.masks import make_identity  # noqa: E402

F32 = mybir.dt.float32
F32R = mybir.dt.float32r
AF = mybir.ActivationFunctionType
ALU = mybir.AluOpType
AX = mybir.AxisListType

B, S, DIM, HIDDEN = 2, 4096, 1024, 2048
T = B * S           # 8192 tokens
E = 8               # experts == cores
TOPK = 2
P = 128
DB = DIM // P       # 8 blocks of 128 along dim
HB = HIDDEN // P    # 16 blocks of 128 along hidden
TSH = T // E        # 1024 tokens per core for the aux router shard
TB = TSH // P       # 8 token blocks per shard

T_TILE = 512        # tokens per MLP pass (fp32 moving-dim max)

N_CORES = 8

# Stash of the most recent device-run results (exec_time_ns etc.) for test
# harnesses; not used by kernel() itself.
LAST_RESULTS = None


def _build(C, mm_dt=F32R, act_fn=None, zero_bias=False):
    """Build the SPMD Bass program for capacity C (multiple of 256).

    zero_bias: both b1 and b2 are all-zero (checked by the caller), so skip
    the bias inputs entirely — drops two slow strided DMAs and the gelu's
    dependency on them."""
    if act_fn is None:
        act_fn = AF.Gelu
    assert C % 256 == 0
    # Full tiles first (PE starts dense and ramps once); remainder last so
    # the final output DMA + drain tail is small.
    rem = C % T_TILE
    widths = [T_TILE] * (C // T_TILE) + ([rem] if rem else [])
    tiles = []
    t0 = 0
    for tw in widths:
        tiles.append((t0, tw))
        t0 += tw
    # Bacc (not raw Bass): its compile() pipeline runs
    # move_matmul_waits_to_ldweights + generate_event_semaphores, which split
    # sync waits down to the 1-wait-per-instruction limit of this walrus.
    nc = bacc.Bacc(None, target_bir_lowering=False)

    xgt = nc.dram_tensor("xgt", [DIM, C], mm_dt, kind="ExternalInput")
    w1t = nc.dram_tensor("w1t", [DIM, HIDDEN], mm_dt, kind="ExternalInput")
    w2t = nc.dram_tensor("w2t", [HIDDEN, DIM], mm_dt, kind="ExternalInput")
    if not zero_bias:
        b1r = nc.dram_tensor("b1r", [P, HB], F32, kind="ExternalInput")
        b2r = nc.dram_tensor("b2r", [P, DB], F32, kind="ExternalInput")
    xst = nc.dram_tensor("xst", [DIM, TSH], mm_dt, kind="ExternalInput")
    rwt = nc.dram_tensor("rwt", [DIM, E], mm_dt, kind="ExternalInput")
    y = nc.dram_tensor("y", [DIM, C], F32, kind="ExternalOutput")
    aux = nc.dram_tensor("aux", [1, 2 * E], F32, kind="ExternalOutput")

    xgt_r = xgt.rearrange("(a p) t -> p a t", p=P)
    w1t_r = w1t.rearrange("(a p) h -> p a h", p=P)
    w2t_r = w2t.rearrange("(a p) d -> p a d", p=P)
    xst_r = xst.rearrange("(a p) t -> p a t", p=P)
    rwt_r = rwt.rearrange("(a p) e -> p a e", p=P)
    y_r = y.rearrange("(a p) t -> p a t", p=P)

    RT = 512               # router token tile (moving dim)
    NRT = TSH // RT        # router tiles per shard

    # All input loads go on the SP HWDGE ring (FIFO per ring, each transfer
    # striped across all 16 SDMA engines) in the order the PE needs them;
    # outputs go on the ACT ring so they never block input loads.
    with TileContext(nc) as tc:
        with (
            tc.tile_pool(name="wsb", bufs=1) as wsb,
            tc.tile_pool(name="csb", bufs=1) as csb,
            tc.tile_pool(name="xsb", bufs=2) as xsb,
            tc.tile_pool(name="hsb", bufs=1) as hsb,
            tc.tile_pool(name="osb", bufs=3) as osb,
            tc.tile_pool(name="rsb", bufs=2) as rsb,
        ):
            if not zero_bias:
                b1s = csb.tile([P, HB], F32)
                nc.sync.dma_start(out=b1s, in_=b1r[:, :])
                b2s = csb.tile([P, DB], F32)
                nc.sync.dma_start(out=b2s, in_=b2r[:, :])

            def load_xg(t0, tw):
                xt = xsb.tile([P, DB, tw], mm_dt, name="xg", tag="xg")
                nc.sync.dma_start(out=xt, in_=xgt_r[:, :, t0 : t0 + tw])
                return [xt[:, db, :] for db in range(DB)]

            def l1(xg, tw):
                h = hsb.tile([P, HB, tw], mm_dt, name="h", tag="h")
                for hb in range(HB):
                    pp = ps1.tile([P, tw], F32, name="pp1", tag="pp1")
                    for db in range(DB):
                        nc.tensor.matmul(
                            pp,
                            lhsT=w1s[db][:, hb * P : (hb + 1) * P],
                            rhs=xg[db],
                            start=(db == 0),
                            stop=(db == DB - 1),
                        )
                    nc.scalar.activation(
                        h[:, hb, :], pp, act_fn,
                        bias=0.0 if zero_bias else b1s[:, hb : hb + 1],
                    )
                return h

            def l2(h, t0, tw):
                for db in range(DB):
                    pp = ps2.tile([P, tw], F32, name="pp2", tag="pp2")
                    for hb in range(HB):
                        nc.tensor.matmul(
                            pp,
                            lhsT=w2s[hb][:, db * P : (db + 1) * P],
                            rhs=h[:, hb, :],
                            start=(hb == 0),
                            stop=(hb == HB - 1),
                        )
                    ot = osb.tile([P, tw], F32, name="ot", tag="ot")
                    if zero_bias:
                        nc.vector.tensor_copy(ot, pp)
                    else:
                        nc.vector.tensor_scalar(
                            ot, pp, b2s[:, db : db + 1], None, op0=ALU.add
                        )
                    nc.scalar.dma_start(out=y_r[:, db, t0 : t0 + tw], in_=ot)

            # tile 0 activations first on the ring, then w1 in the exact
            # order layer 1 consumes it (hb block 0 first at fine grain).
            xg0 = load_xg(*tiles[0])
            w1s = [wsb.tile([P, HIDDEN], mm_dt, name=f"w1s{db}") for db in range(DB)]
            QW = 512
            for q in range(HIDDEN // QW):
                cs = slice(q * QW, (q + 1) * QW)
                for db in range(DB):
                    nc.sync.dma_start(out=w1s[db][:, cs], in_=w1t_r[:, db, cs])

            # router constants + state (aux phase B accumulators)
            ones = csb.tile([P, 1], F32)
            nc.vector.memset(ones, 1.0)
            ident = csb.tile([E, E], F32)
            make_identity(nc, ident)
            rws = csb.tile([P, DB, E], mm_dt)
            nc.sync.dma_start(out=rws, in_=rwt_r)
            lgs = csb.tile([P, TB, E], F32)
            accP = csb.tile([P, E], F32)
            nc.vector.memset(accP, 0.0)
            accF = csb.tile([P, E], F32)
            nc.vector.memset(accF, 0.0)

            with tc.tile_pool(name="ps1", bufs=4, space="PSUM") as ps1:
                h0 = l1(xg0, tiles[0][1])

                # ---- aux router phase A: logits + transposes (PE), stashed
                # to SBUF; xsT loads queue behind w1 on the ring. ----
                with (
                    tc.tile_pool(name="rps", bufs=2, space="PSUM") as rps,
                    tc.tile_pool(name="rtps", bufs=2, space="PSUM") as rtps,
                ):
                    nblk = 0
                    for rt in range(NRT):
                        xst_tile = xsb.tile(
                            [P, DB, RT], mm_dt, name="xs", tag="xg"
                        )
                        nc.sync.dma_start(
                            out=xst_tile,
                            in_=xst_r[:, :, rt * RT : (rt + 1) * RT],
                        )
                        xs = [xst_tile[:, db, :] for db in range(DB)]
                        ltp = rps.tile([E, RT], F32, name="ltp")
                        for db in range(DB):
                            nc.tensor.matmul(
                                ltp,
                                lhsT=rws[:, db, :],
                                rhs=xs[db],
                                start=(db == 0),
                                stop=(db == DB - 1),
                            )
                        lt = rsb.tile([E, RT], F32, name="lt")
                        nc.vector.tensor_copy(lt, ltp)
                        for j in range(RT // P):
                            lg = rtps.tile([P, E], F32, name="lg")
                            nc.tensor.transpose(
                                lg, lt[:, j * P : (j + 1) * P], ident
                            )
                            nc.vector.tensor_copy(lgs[:, nblk, :], lg)
                            nblk += 1

                # w2, in the half-major order layer 2 consumes it.
                w2s = [wsb.tile([P, DIM], mm_dt, name=f"w2s{hb}") for hb in range(HB)]
                for half in range(2):
                    cs = slice(half * (DIM // 2), (half + 1) * (DIM // 2))
                    for hb in range(HB):
                        nc.sync.dma_start(out=w2s[hb][:, cs], in_=w2t_r[:, hb, cs])

                with (
                    tc.tile_pool(name="ps2", bufs=3, space="PSUM") as ps2,
                    tc.tile_pool(name="accps", bufs=1, space="PSUM") as accps,
                ):
                    l2(h0, tiles[0][0], tiles[0][1])

                    # ---- aux router phase B: softmax/top-2 on DVE/ACT,
                    # accumulated on DVE; overlaps the MLP's PE stream. ----
                    for blk in range(TB):
                        lg = lgs[:, blk, :]
                        mx = rsb.tile([P, 8], F32, name="mx")
                        nc.vector.max(out=mx, in_=lg)
                        nm = rsb.tile([P, 1], F32, name="nm")
                        nc.vector.tensor_scalar_mul(nm, mx[:, 0:1], -1.0)
                        ex = rsb.tile([P, E], F32, name="ex")
                        nc.scalar.activation(ex, lg, AF.Exp, bias=nm)
                        sm = rsb.tile([P, 1], F32, name="sm")
                        nc.vector.reduce_sum(sm, ex, axis=AX.X)
                        rs = rsb.tile([P, 1], F32, name="rs")
                        nc.vector.reciprocal(rs, sm)
                        pr = rsb.tile([P, E], F32, name="pr")
                        nc.vector.tensor_mul(pr, ex, rs.to_broadcast([P, E]))
                        nc.vector.tensor_add(accP, accP, pr)
                        msk = rsb.tile([P, E], F32, name="msk")
                        nc.vector.tensor_tensor(
                            msk,
                            lg,
                            mx[:, 1:2].to_broadcast([P, E]),
                            op=ALU.is_ge,
                        )
                        nc.vector.tensor_add(accF, accF, msk)
                    pPF = accps.tile([1, 2 * E], F32, name="pPF")
                    nc.tensor.matmul(
                        pPF[:, 0:E], lhsT=ones, rhs=accP,
                        start=True, stop=True,
                    )
                    nc.tensor.matmul(
                        pPF[:, E : 2 * E], lhsT=ones, rhs=accF,
                        start=True, stop=True, skip_group_check=True,
                    )
                    auxs = rsb.tile([1, 2 * E], F32, name="auxs")
                    nc.vector.tensor_copy(auxs, pPF)
                    nc.scalar.dma_start(out=aux[:, :], in_=auxs)

                    for t0, tw in tiles[1:]:
                        xg = load_xg(t0, tw)
                        h = l1(xg, tw)
                        l2(h, t0, tw)

    nc.finalize()
    return nc


def _route_host(xf, router_w):
    """Host top-2 routing (the sharding decision). Matches jax.lax.top_k
    tie-breaking (lowest index wins)."""
    logits = xf @ router_w.T  # [T, E] f32
    t_idx = np.arange(logits.shape[0])
    i1 = np.argmax(logits, axis=1)
    l1 = logits[t_idx, i1]
    lm = logits.copy()
    lm[t_idx, i1] = -np.inf
    i2 = np.argmax(lm, axis=1)
    l2 = logits[t_idx, i2]
    # softmax over the two selected logits (l1 >= l2)
    e2 = np.exp((l2 - l1).astype(np.float32))
    g1 = (1.0 / (1.0 + e2)).astype(np.float32)
    g2 = (e2 / (1.0 + e2)).astype(np.float32)
    return i1, i2, g1, g2


def kernel(x, router_w, w1, b1, w2, b2):
    global LAST_RESULTS
    x = np.asarray(x, dtype=np.float32)
    router_w = np.asarray(router_w, dtype=np.float32)
    w1 = np.asarray(w1, dtype=np.float32)
    b1 = np.asarray(b1, dtype=np.float32)
    w2 = np.asarray(w2, dtype=np.float32)
    b2 = np.asarray(b2, dtype=np.float32)

    xf = x.reshape(T, DIM)
    i1, i2, g1, g2 = _route_host(xf, router_w)

    idx = []
    gates = []
    for e in range(E):
        sel1 = i1 == e
        sel2 = i2 == e
        ie = np.where(sel1 | sel2)[0]
        ge = np.where(sel1[ie], g1[ie], g2[ie]).astype(np.float32)
        idx.append(ie)
        gates.append(ge)

    max_n = max(len(ie) for ie in idx)
    C = max(256, ((max_n + 255) // 256) * 256)

    mm_dt = F32 if os.environ.get("MOE_MM_F32") else F32R
    zero_bias = not (b1.any() or b2.any())
    nc = _build(C, mm_dt=mm_dt, zero_bias=zero_bias)

    rwt = np.ascontiguousarray(router_w.T)
    in_maps = []
    for e in range(E):
        xgt = np.zeros((DIM, C), dtype=np.float32)
        ie = idx[e]
        if len(ie):
            xgt[:, : len(ie)] = xf[ie].T
        im = {
            "xgt": xgt,
            "w1t": np.ascontiguousarray(w1[e].T),
            "w2t": np.ascontiguousarray(w2[e].T),
            "xst": np.ascontiguousarray(xf[e * TSH : (e + 1) * TSH].T),
            "rwt": rwt,
        }
        if not zero_bias:
            im["b1r"] = np.ascontiguousarray(b1[e].reshape(HB, P).T)
            im["b2r"] = np.ascontiguousarray(b2[e].reshape(DB, P).T)
        in_maps.append(im)

    trace = bool(os.environ.get("BASS_TRACE"))
    res = run_bass_kernel_spmd(
        nc,
        in_maps,
        core_ids=list(range(N_CORES)),
        trace=trace,
    )
    LAST_RESULTS = res

    out = np.zeros((T, DIM), dtype=np.float32)
    p_sum = np.zeros(E, dtype=np.float64)
    f_cnt = np.zeros(E, dtype=np.float64)
    for e in range(E):
        r = res.results[e]
        ie = idx[e]
        ye = r["y"]
        out[ie] += gates[e][:, None] * ye[:, : len(ie)].T
        p_sum += r["aux"][0, :E].astype(np.float64)
        f_cnt += r["aux"][0, E:].astype(np.float64)

    P_i = p_sum / T
    f_i = f_cnt / (T * TOPK)
    aux_loss = np.float32(E * np.sum(f_i * P_i))

    return out.reshape(B, S, DIM), aux_loss


# revision 26
# speedup vs baseline: 1.0452x; 1.0140x over previous
"""Trainium2 Bass kernel for an 8-expert top-2 MoE block (dim=1024, hidden=2048).

Strategy (expert-parallel, per the sharding hint):
  - Host computes the router top-2 dispatch (a sharding decision) and compacts
    each expert's tokens into a padded [dim, C] buffer, transposed so the
    device can run weight-stationary matmuls without any on-chip transposes.
  - Core e holds expert e's weights resident in SBUF and runs the 2-layer
    GELU MLP over its gathered tokens: x -> w1 -> gelu -> w2 (+biases).
  - The router/softmax statistics needed for the aux load-balancing loss are
    computed on-device data-parallel: core e processes token shard e
    (1024 tokens), producing per-expert softmax-prob sums and top-2 counts.
  - Host scatters the gated expert outputs back to token order and reduces
    the aux statistics to the scalar aux loss.
"""

import os
import sys
import types

import numpy as np

# Make sure jax can see the axon-tunneled NeuronCores if it hasn't loaded yet.
_jp = os.environ.get("JAX_PLATFORMS", "")
if "axon" not in _jp and "jax" not in sys.modules:
    os.environ["JAX_PLATFORMS"] = "axon,cpu"


def _install_axon_hook_shim():
    """Provide antenv.axon_hooks if the image's antenv predates it, so
    run_bass_kernel_spmd(trace=True) works instead of crashing."""
    try:
        import antenv.axon_hooks  # noqa: F401
        return
    except Exception:
        pass
    try:
        import antenv
    except Exception:
        return
    m = types.ModuleType("antenv.axon_hooks")
    m._h = None

    def set_axon_ntff_profile_hook(h):
        m._h = h

    def get_axon_ntff_profile_hook():
        return m._h

    m.set_axon_ntff_profile_hook = set_axon_ntff_profile_hook
    m.get_axon_ntff_profile_hook = get_axon_ntff_profile_hook
    sys.modules["antenv.axon_hooks"] = m
    antenv.axon_hooks = m
    try:
        from trn_agent_boot.trn_boot import _ntff_profile_via_ctypes

        h = _ntff_profile_via_ctypes("/opt/axon/libaxon_pjrt.so")
        if h is not None:
            m._h = h
    except Exception:
        pass


_install_axon_hook_shim()

import concourse.bass as bass  # noqa: E402
import concourse.mybir as mybir  # noqa: E402
from concourse import bacc  # noqa: E402
from concourse.tile import TileContext  # noqa: E402
from concourse.bass_utils import run_bass_kernel_spmd  # noqa: E402
from concourse.masks import make_identity  # noqa: E402

F32 = mybir.dt.float32
F32R = mybir.dt.float32r
AF = mybir.ActivationFunctionType
ALU = mybir.AluOpType
AX = mybir.AxisListType

B, S, DIM, HIDDEN = 2, 4096, 1024, 2048
T = B * S           # 8192 tokens
E = 8               # experts == cores
TOPK = 2
P = 128
DB = DIM // P       # 8 blocks of 128 along dim
HB = HIDDEN // P    # 16 blocks of 128 along hidden
TSH = T // E        # 1024 tokens per core for the aux router shard
TB = TSH // P       # 8 token blocks per shard

T_TILE = 512        # tokens per MLP pass (fp32 moving-dim max)

N_CORES = 8

# Stash of the most recent device-run results (exec_time_ns etc.) for test
# harnesses; not used by kernel() itself.
LAST_RESULTS = None


def _build(C, mm_dt=F32R, act_fn=None, zero_bias=False):
    """Build the SPMD Bass program for capacity C (multiple of 256).

    zero_bias: both b1 and b2 are all-zero (checked by the caller), so skip
    the bias inputs entirely.

    Router statistics for the aux loss are computed over the gathered tokens
    already resident for the MLP (each real token is gathered on exactly two
    cores, so the host halves the cross-core sums); `vld` masks out padding
    columns."""
    if act_fn is None:
        act_fn = AF.Gelu
    assert C % 256 == 0
    # Full tiles first (PE starts dense and ramps once); remainder last so
    # the final output DMA + drain tail is small.
    rem = C % T_TILE
    widths = [T_TILE] * (C // T_TILE) + ([rem] if rem else [])
    tiles = []
    t0 = 0
    for tw in widths:
        tiles.append((t0, tw))
        t0 += tw
    NBLK = C // P
    # Bacc (not raw Bass): its compile() pipeline runs
    # move_matmul_waits_to_ldweights + generate_event_semaphores, which split
    # sync waits down to the 1-wait-per-instruction limit of this walrus.
    nc = bacc.Bacc(None, target_bir_lowering=False)

    xgt = nc.dram_tensor("xgt", [DIM, C], mm_dt, kind="ExternalInput")
    w1t = nc.dram_tensor("w1t", [DIM, HIDDEN], mm_dt, kind="ExternalInput")
    w2t = nc.dram_tensor("w2t", [HIDDEN, DIM], mm_dt, kind="ExternalInput")
    if not zero_bias:
        b1r = nc.dram_tensor("b1r", [P, HB], F32, kind="ExternalInput")
        b2r = nc.dram_tensor("b2r", [P, DB], F32, kind="ExternalInput")
    rwt = nc.dram_tensor("rwt", [DIM, E], mm_dt, kind="ExternalInput")
    vld = nc.dram_tensor("vld", [C, 1], F32, kind="ExternalInput")
    y = nc.dram_tensor("y", [DIM, C], F32, kind="ExternalOutput")
    aux = nc.dram_tensor("aux", [1, 2 * E], F32, kind="ExternalOutput")

    xgt_r = xgt.rearrange("(a p) t -> p a t", p=P)
    w1t_r = w1t.rearrange("(a p) h -> p a h", p=P)
    w2t_r = w2t.rearrange("(a p) d -> p a d", p=P)
    rwt_r = rwt.rearrange("(a p) e -> p a e", p=P)
    vld_r = vld.rearrange("(a p) x -> p a x", p=P)
    y_r = y.rearrange("(a p) t -> p a t", p=P)

    # All input loads go on the SP HWDGE ring (FIFO per ring, each transfer
    # striped across all 16 SDMA engines) in the order the PE needs them;
    # outputs go on the ACT ring so they never block input loads.
    with TileContext(nc) as tc:
        with (
            tc.tile_pool(name="wsb", bufs=1) as wsb,
            tc.tile_pool(name="csb", bufs=1) as csb,
            tc.tile_pool(name="xsb", bufs=2) as xsb,
            tc.tile_pool(name="hsb", bufs=1) as hsb,
            tc.tile_pool(name="osb", bufs=3) as osb,
            tc.tile_pool(name="rsb", bufs=2) as rsb,
            tc.tile_pool(name="ps1", bufs=4, space="PSUM") as ps1,
            tc.tile_pool(name="ps2", bufs=2, space="PSUM") as ps2,
            tc.tile_pool(name="rps", bufs=1, space="PSUM") as rps,
            tc.tile_pool(name="rtps", bufs=1, space="PSUM") as rtps,
        ):
            def load_xg(t0, tw):
                xt = xsb.tile([P, DB, tw], mm_dt, name="xg", tag="xg")
                nc.sync.dma_start(out=xt, in_=xgt_r[:, :, t0 : t0 + tw])
                return xt

            def l1(xg, tw):
                h = hsb.tile([P, HB, tw], mm_dt, name="h", tag="h")
                for hb in range(HB):
                    pp = ps1.tile([P, tw], F32, name="pp1", tag="pp1")
                    for db in range(DB):
                        nc.tensor.matmul(
                            pp,
                            lhsT=w1s[db][:, hb * P : (hb + 1) * P],
                            rhs=xg[:, db, :],
                            start=(db == 0),
                            stop=(db == DB - 1),
                        )
                    nc.scalar.activation(
                        h[:, hb, :], pp, act_fn,
                        bias=0.0 if zero_bias else b1s[:, hb : hb + 1],
                    )
                return h

            def router(xg, t0, tw):
                # logits^T for this tile's tokens from the resident xg tile,
                # transposed back per 128-token block; probs/top-2 masked by
                # validity and accumulated on DVE.
                ltp = rps.tile([E, tw], F32, name="ltp", tag="ltp")
                for db in range(DB):
                    nc.tensor.matmul(
                        ltp,
                        lhsT=rws[:, db, :],
                        rhs=xg[:, db, :],
                        start=(db == 0),
                        stop=(db == DB - 1),
                    )
                lt = rsb.tile([E, tw], F32, name="lt", tag="lt")
                nc.vector.tensor_copy(lt, ltp)
                for j in range(tw // P):
                    blk = t0 // P + j
                    lg = rtps.tile([P, E], F32, name="lg")
                    nc.tensor.transpose(lg, lt[:, j * P : (j + 1) * P], ident)
                    lgs = rsb.tile([P, E], F32, name="lgs")
                    nc.vector.tensor_copy(lgs, lg)
                    mx = rsb.tile([P, 8], F32, name="mx")
                    nc.vector.max(out=mx, in_=lgs)
                    nm = rsb.tile([P, 1], F32, name="nm")
                    nc.vector.tensor_scalar_mul(nm, mx[:, 0:1], -1.0)
                    ex = rsb.tile([P, E], F32, name="ex")
                    nc.scalar.activation(ex, lgs, AF.Exp, bias=nm)
                    sm = rsb.tile([P, 1], F32, name="sm")
                    nc.vector.reduce_sum(sm, ex, axis=AX.X)
                    rs = rsb.tile([P, 1], F32, name="rs")
                    nc.vector.reciprocal(rs, sm)
                    # validity-weighted probs: rs * vld  (padding rows -> 0)
                    rv = rsb.tile([P, 1], F32, name="rv")
                    nc.vector.tensor_mul(rv, rs, vlds[:, blk : blk + 1])
                    pr = rsb.tile([P, E], F32, name="pr")
                    nc.vector.tensor_mul(pr, ex, rv.to_broadcast([P, E]))
                    nc.vector.tensor_add(accP, accP, pr)
                    msk = rsb.tile([P, E], F32, name="msk")
                    nc.vector.tensor_tensor(
                        msk, lgs, mx[:, 1:2].to_broadcast([P, E]), op=ALU.is_ge
                    )
                    mskv = rsb.tile([P, E], F32, name="mskv")
                    nc.vector.tensor_mul(
                        mskv, msk, vlds[:, blk : blk + 1].to_broadcast([P, E])
                    )
                    nc.vector.tensor_add(accF, accF, mskv)

            def l2(h, t0, tw):
                for db in range(DB):
                    pp = ps2.tile([P, tw], F32, name="pp2", tag="pp2")
                    for hb in range(HB):
                        nc.tensor.matmul(
                            pp,
                            lhsT=w2s[hb][:, db * P : (db + 1) * P],
                            rhs=h[:, hb, :],
                            start=(hb == 0),
                            stop=(hb == HB - 1),
                        )
                    ot = osb.tile([P, tw], F32, name="ot", tag="ot")
                    if zero_bias:
                        nc.vector.tensor_copy(ot, pp)
                    else:
                        nc.vector.tensor_scalar(
                            ot, pp, b2s[:, db : db + 1], None, op0=ALU.add
                        )
                    nc.scalar.dma_start(out=y_r[:, db, t0 : t0 + tw], in_=ot)

            # tile 0 activations first on the ring, then w1 in the exact
            # quarter-major order layer 1 consumes it.
            xg0 = load_xg(*tiles[0])
            w1s = [wsb.tile([P, HIDDEN], mm_dt, name=f"w1s{db}") for db in range(DB)]
            QW = 512
            for q in range(HIDDEN // QW):
                cs = slice(q * QW, (q + 1) * QW)
                for db in range(DB):
                    nc.sync.dma_start(out=w1s[db][:, cs], in_=w1t_r[:, db, cs])
            if not zero_bias:
                b1s = csb.tile([P, HB], F32)
                nc.sync.dma_start(out=b1s, in_=b1r[:, :])
                b2s = csb.tile([P, DB], F32)
                nc.sync.dma_start(out=b2s, in_=b2r[:, :])
            # router constants + aux accumulators
            ones = csb.tile([P, 1], F32)
            nc.vector.memset(ones, 1.0)
            ident = csb.tile([E, E], F32)
            make_identity(nc, ident)
            rws = csb.tile([P, DB, E], mm_dt)
            nc.sync.dma_start(out=rws, in_=rwt_r)
            vlds = csb.tile([P, NBLK], F32)
            nc.sync.dma_start(out=vlds, in_=vld_r[:, :, 0])
            accP = csb.tile([P, E], F32)
            nc.vector.memset(accP, 0.0)
            accF = csb.tile([P, E], F32)
            nc.vector.memset(accF, 0.0)

            h0 = l1(xg0, tiles[0][1])
            router(xg0, *tiles[0])

            # w2, in the half-major order layer 2 consumes it.
            w2s = [wsb.tile([P, DIM], mm_dt, name=f"w2s{hb}") for hb in range(HB)]
            for half in range(2):
                cs = slice(half * (DIM // 2), (half + 1) * (DIM // 2))
                for hb in range(HB):
                    nc.sync.dma_start(out=w2s[hb][:, cs], in_=w2t_r[:, hb, cs])

            l2(h0, tiles[0][0], tiles[0][1])
            for t0, tw in tiles[1:]:
                xg = load_xg(t0, tw)
                h = l1(xg, tw)
                router(xg, t0, tw)
                l2(h, t0, tw)

            # final cross-partition sums -> aux output
            pPF = ps2.tile([P, T_TILE], F32, name="pPF", tag="pp2")
            nc.tensor.matmul(
                pPF[0:1, 0:E], lhsT=ones, rhs=accP, start=True, stop=True
            )
            nc.tensor.matmul(
                pPF[0:1, E : 2 * E], lhsT=ones, rhs=accF,
                start=True, stop=True, skip_group_check=True,
            )
            auxs = rsb.tile([1, 2 * E], F32, name="auxs")
            nc.vector.tensor_copy(auxs, pPF[0:1, 0 : 2 * E])
            nc.scalar.dma_start(out=aux[:, :], in_=auxs)

    nc.finalize()
    return nc


def _route_host(xf, router_w):
    """Host top-2 routing (the sharding decision). Matches jax.lax.top_k
    tie-breaking (lowest index wins)."""
    logits = xf @ router_w.T  # [T, E] f32
    t_idx = np.arange(logits.shape[0])
    i1 = np.argmax(logits, axis=1)
    l1 = logits[t_idx, i1]
    lm = logits.copy()
    lm[t_idx, i1] = -np.inf
    i2 = np.argmax(lm, axis=1)
    l2 = logits[t_idx, i2]
    # softmax over the two selected logits (l1 >= l2)
    e2 = np.exp((l2 - l1).astype(np.float32))
    g1 = (1.0 / (1.0 + e2)).astype(np.float32)
    g2 = (e2 / (1.0 + e2)).astype(np.float32)
    return i1, i2, g1, g2


def kernel(x, router_w, w1, b1, w2, b2):
    global LAST_RESULTS
    x = np.asarray(x, dtype=np.float32)
    router_w = np.asarray(router_w, dtype=np.float32)
    w1 = np.asarray(w1, dtype=np.float32)
    b1 = np.asarray(b1, dtype=np.float32)
    w2 = np.asarray(w2, dtype=np.float32)
    b2 = np.asarray(b2, dtype=np.float32)

    xf = x.reshape(T, DIM)
    i1, i2, g1, g2 = _route_host(xf, router_w)

    idx = []
    gates = []
    for e in range(E):
        sel1 = i1 == e
        sel2 = i2 == e
        ie = np.where(sel1 | sel2)[0]
        ge = np.where(sel1[ie], g1[ie], g2[ie]).astype(np.float32)
        idx.append(ie)
        gates.append(ge)

    max_n = max(len(ie) for ie in idx)
    C = max(256, ((max_n + 255) // 256) * 256)

    mm_dt = F32 if os.environ.get("MOE_MM_F32") else F32R
    zero_bias = not (b1.any() or b2.any())
    nc = _build(C, mm_dt=mm_dt, zero_bias=zero_bias)

    rwt = np.ascontiguousarray(router_w.T)
    in_maps = []
    for e in range(E):
        xgt = np.zeros((DIM, C), dtype=np.float32)
        ie = idx[e]
        if len(ie):
            xgt[:, : len(ie)] = xf[ie].T
        vld = np.zeros((C, 1), dtype=np.float32)
        vld[: len(ie)] = 1.0
        im = {
            "xgt": xgt,
            "w1t": np.ascontiguousarray(w1[e].T),
            "w2t": np.ascontiguousarray(w2[e].T),
            "rwt": rwt,
            "vld": vld,
        }
        if not zero_bias:
            im["b1r"] = np.ascontiguousarray(b1[e].reshape(HB, P).T)
            im["b2r"] = np.ascontiguousarray(b2[e].reshape(DB, P).T)
        in_maps.append(im)

    trace = bool(os.environ.get("BASS_TRACE"))
    res = run_bass_kernel_spmd(
        nc,
        in_maps,
        core_ids=list(range(N_CORES)),
        trace=trace,
    )
    LAST_RESULTS = res

    out = np.zeros((T, DIM), dtype=np.float32)
    p_sum = np.zeros(E, dtype=np.float64)
    f_cnt = np.zeros(E, dtype=np.float64)
    for e in range(E):
        r = res.results[e]
        ie = idx[e]
        ye = r["y"]
        out[ie] += gates[e][:, None] * ye[:, : len(ie)].T
        p_sum += r["aux"][0, :E].astype(np.float64)
        f_cnt += r["aux"][0, E:].astype(np.float64)

    # each real token is gathered on exactly TOPK cores -> halve the sums
    P_i = p_sum / (TOPK * T)
    f_i = f_cnt / (TOPK * T * TOPK)
    aux_loss = np.float32(E * np.sum(f_i * P_i))

    return out.reshape(B, S, DIM), aux_loss
